# revision 23
# baseline (speedup 1.0000x reference)
"""Trainium2 Bass kernel for nn_BasicTT: 3 TT layers + linear head.

Strategy
--------
Data-parallel over batch: 512 samples -> 8 cores x 64 samples. Weights
replicated. Each TT layer (cores c1..c6, mid rank R) is evaluated via the
exact Kronecker 3+3 merge

    y[(i1,i2)] = sum_r A_r @ X @ B_r^T,   A[i1,u,R], B[R,i2,v], X=[u,v]

which is two TensorE passes per layer and needs no on-chip transposes:
 pass1 (per sample): stationary lhsT = X chunk [u(K), v(M)], moving
   rhs = A^T stacked [u, (r,i1)]  ->  W[v, (r,i1)]      (PE contracts u)
 pass2 (per batch-tile): stationary lhsT = B^T [v, i2], moving
   rhs = W [v, (s,i1)] per r, accumulated in PSUM       (PE contracts v,r)
Layer biases are folded in as one extra identity matmul into the same PSUM
accumulation; ReLU is fused into the PSUM->SBUF copy on ScalarE.
Layer outputs alternate orientation ([i2, i1] vs [i1, i2]) which exactly
matches the next layer's expected (v-on-partitions / u-on-partitions) input,
so the whole network chains with zero data reshuffling.

Compute dtype is bf16 (f32 PSUM accumulation); verified end-to-end error vs
the f32 reference is ~6e-3 relative.
"""

import sys

sys.path.insert(0, "/opt/trn_rl_repo")

import numpy as np
import ml_dtypes

import concourse.bass as bass
import concourse.bacc as bacc
import concourse.mybir as mybir
from concourse.tile import TileContext
from concourse.bass_utils import run_bass_kernel_spmd

BF16 = ml_dtypes.bfloat16
N_CORES = 8
BATCH = 512
B_CORE = BATCH // N_CORES          # 64 samples per core
BT = 8                             # batch sub-tile (samples per inner iteration)
N_BT = B_CORE // BT

FEAT = 65536                       # 4*8*8*4*8*8
U1, V1, R1, I1, J1 = 256, 256, 3, 64, 64     # TT1: u,v,midrank, m_first, m_last
U2, V2, R2, I2, J2 = 64, 64, 2, 16, 16       # TT2
U3, V3, R3, I3, J3 = 16, 16, 2, 8, 8         # TT3


def _merge_first3(cores):
    """cores[0..2] -> A[i1, u, R] (m-major / n-major flattening)."""
    c1, c2, c3 = [np.asarray(c, np.float32) for c in cores]
    A = np.einsum("amnr,rpqs,sxyt->mpxnqyt", c1, c2, c3)
    return A.reshape(
        c1.shape[1] * c2.shape[1] * c3.shape[1],
        c1.shape[2] * c2.shape[2] * c3.shape[2],
        c3.shape[3],
    )


def _merge_last3(cores):
    """cores[3..5] -> B[R, i2, v]."""
    c4, c5, c6 = [np.asarray(c, np.float32) for c in cores]
    B = np.einsum("amnr,rpqs,sxyt->ampxnqy", c4, c5, c6)
    return B.reshape(
        c4.shape[0],
        c4.shape[1] * c5.shape[1] * c6.shape[1],
        c4.shape[2] * c5.shape[2] * c6.shape[2],
    )


def _prep_weights(cores1, bias1, cores2, bias2, cores3, bias3, w_lin, b_lin):
    """Host-side merge + layout. All outputs bf16 (except noted)."""
    w = {}

    # ---- TT1 ----
    A1 = _merge_first3(cores1[:3])            # [64, 256, 3]
    B1 = _merge_last3(cores1[3:])             # [3, 64, 256]
    # pass1 moving operand: a1t[c][p, r*64+i1] = A1[i1, u=2p+c, r]
    a1t = A1.transpose(1, 2, 0).reshape(U1, R1 * I1)     # [u, (r,i1)]
    a1t = np.stack([a1t[0::2, :], a1t[1::2, :]])         # [2, 128, 192]
    w["a1t"] = a1t.astype(BF16)
    # pass2 stationary: b1t[vc][v_lo, r*64+i2] = B1[r, i2, v=vc*128+v_lo]
    b1t = B1.transpose(2, 0, 1).reshape(V1, R1 * J1)     # [v, (r,i2)]
    w["b1t"] = np.stack([b1t[:128], b1t[128:]]).astype(BF16)  # [2, 128, 192]
    # bias1 in output layout [i2, i1], replicated BT times along free-major s
    b1 = np.asarray(bias1, np.float32).reshape(I1, J1).T  # [i2, i1]
    w["bias1rep"] = np.tile(b1[:, None, :], (1, BT, 1)).reshape(J1, BT * I1).astype(BF16)

    # ---- TT2 (input arrives flipped: [v2, u2] on partitions) ----
    A2 = _merge_first3(cores2[:3])            # [16, 64, 2]
    B2 = _merge_last3(cores2[3:])             # [2, 16, 64]
    # pass1 moving: b2stack[v2, (r,j2)] = B2[r, j2, v2]
    w["b2stack"] = B2.transpose(2, 0, 1).reshape(V2, R2 * J2).astype(BF16)  # [64, 32]
    # pass2 stationary: a2t[r][u2, j1] = A2[j1, u2, r]
    w["a2t"] = A2.transpose(2, 1, 0).astype(BF16)        # [2, 64, 16]
    b2 = np.asarray(bias2, np.float32).reshape(16, 16)   # [j1, j2] j1-major
    w["bias2rep"] = np.tile(b2[:, None, :], (1, BT, 1)).reshape(16, BT * 16).astype(BF16)

    # ---- TT3 (input arrives normal: [u3, v3-ish free]) ----
    A3 = _merge_first3(cores3[:3])            # [8, 16, 2]
    B3 = _merge_last3(cores3[3:])             # [2, 8, 16]
    # pass1 moving: a3stack[u3, (r,i1)] = A3[i1, u3, r]
    w["a3stack"] = A3.transpose(1, 2, 0).reshape(U3, R3 * I3).astype(BF16)  # [16, 16]
    # pass2 stationary: b3t[r][v3, i2] = B3[r, i2, v3]
    w["b3t"] = B3.transpose(2, 0, 1).astype(BF16)        # [16, 2, 8] -> index [v3, r, i2]
    b3 = np.asarray(bias3, np.float32).reshape(I3 * J3)  # flat (i1,i2) i1-major
    b3 = b3.reshape(I3, J3).T                            # [i2, i1]
    w["bias3rep"] = np.tile(b3[:, None, :], (1, BT, 1)).reshape(J3, BT * I3).astype(BF16)

    # ---- linear head ----
    wl = np.asarray(w_lin, np.float32).reshape(10, I3, J3)  # [o, i1, i2]
    w["wlt"] = wl.transpose(2, 1, 0).astype(BF16)        # [i2=8, i1=8, 10]
    w["blin"] = np.asarray(b_lin, np.float32).reshape(1, 10).astype(BF16)

    # identities for the bias matmuls, ones row for the head bias
    w["eye64"] = np.eye(64, dtype=np.float32).astype(BF16)
    w["eye16"] = np.eye(16, dtype=np.float32).astype(BF16)
    w["eye8"] = np.eye(8, dtype=np.float32).astype(BF16)
    w["ones64"] = np.ones((1, 64), dtype=np.float32).astype(BF16)

    # pack everything into a single [128, WPACK_COLS] blob: one DMA at start
    pack = np.zeros((128, WPACK_COLS), dtype=BF16)
    def put2d(arr2d, name):
        rows, cols, off = WPACK_SLOTS[name]
        assert arr2d.shape == (rows, cols), (name, arr2d.shape)
        pack[:rows, off:off + cols] = arr2d
    put2d(np.concatenate([w["a1t"][0], w["a1t"][1]], axis=1), "a1t")
    put2d(np.concatenate([w["b1t"][0], w["b1t"][1]], axis=1), "b1t")
    put2d(w["bias1rep"], "bias1rep")
    put2d(w["b2stack"], "b2stack")
    a2t2d = np.concatenate([w["a2t"][0], w["a2t"][1]], axis=1)
    put2d(a2t2d, "a2t")
    pack[64:128, 1312:1344] = a2t2d          # copy for the odd-half row group
    put2d(w["bias2rep"], "bias2rep")
    put2d(w["a3stack"], "a3stack")
    b3t2d = w["b3t"].reshape(16, 16)
    put2d(b3t2d, "b3t")
    for k in (1, 2, 3):                      # copies for row groups 32/64/96
        pack[32 * k:32 * k + 16, 1488:1504] = b3t2d
    put2d(w["bias3rep"], "bias3rep")
    put2d(w["wlt"].reshape(8, 80), "wlt")
    put2d(w["blin"], "blin")
    put2d(w["eye64"], "eye64")
    put2d(w["eye16"], "eye16")
    put2d(w["eye8"], "eye8")
    put2d(w["ones64"], "ones64")
    return {"wpack": pack}


# name -> (rows, cols, col_offset) in the packed weight blob
WPACK_SLOTS = {
    "a1t": (128, 384, 0), "b1t": (128, 384, 384), "bias1rep": (64, 512, 768),
    "b2stack": (64, 32, 1280), "a2t": (64, 32, 1312), "bias2rep": (16, 128, 1344),
    "a3stack": (16, 16, 1472), "b3t": (16, 16, 1488), "bias3rep": (8, 64, 1504),
    "wlt": (8, 80, 1568), "blin": (1, 10, 1648), "eye64": (64, 64, 1664),
    "eye16": (16, 16, 1728), "eye8": (8, 8, 1744), "ones64": (1, 64, 1760),
}
WPACK_COLS = 1824

_nc_cache = None


def _build_nc():
    f32 = mybir.dt.float32
    bf = mybir.dt.bfloat16
    nc = bacc.Bacc()

    x_ext = nc.declare_dram_parameter("x", [B_CORE, FEAT], bf, isOutput=False)
    wp_ext = nc.declare_dram_parameter("wpack", [128, WPACK_COLS], bf, isOutput=False)
    y_ext = nc.declare_dram_parameter("y", [10, B_CORE], f32, isOutput=True)

    with TileContext(nc) as tc:
        with (
            tc.tile_pool(name="consts", bufs=1) as cpool,
            tc.tile_pool(name="xt", bufs=3) as xpool,
            tc.tile_pool(name="w1", bufs=2) as w1pool,
            tc.tile_pool(name="act", bufs=2) as apool,
            tc.tile_pool(name="h3", bufs=1) as h3pool,
            tc.tile_pool(name="out", bufs=1) as opool,
            tc.tile_pool(name="ps_w", bufs=4, space="PSUM") as ps_w,
            tc.tile_pool(name="ps_y", bufs=2, space="PSUM") as ps_y,
            tc.tile_pool(name="ps_s", bufs=2, space="PSUM") as ps_s,
        ):
            # -- load all weights with a single DMA on the SWDGE path, so
            # -- it never queues behind the HWDGE x loads --
            wsb = cpool.tile([128, WPACK_COLS], bf)
            # a1t first: TT1 pass1 only needs cols 0:384, so the first matmul
            # can start as soon as this small DMA lands
            nc.gpsimd.dma_start(out=wsb[:, :384], in_=wp_ext[:, :384])
            nc.gpsimd.dma_start(out=wsb[:, 384:], in_=wp_ext[:, 384:])
            sb = {
                name: wsb[:rows, off:off + cols]
                for name, (rows, cols, off) in WPACK_SLOTS.items()
            }

            h3all = h3pool.tile([8, B_CORE * I3], bf)   # [i2'', (b, i1'')]

            for ibt in range(N_BT):
                # ---- load BT samples, cast f32 -> bf16 on the fly ----
                xt = xpool.tile([128, BT * 512], bf)     # free = (s, u_lo=2, v=256)
                # per-sample DMAs so pass1 for sample s starts as soon as its
                # 128KB lands instead of waiting for the full 1MB tile
                for s in range(BT):
                    eng = nc.sync if s % 2 == 0 else nc.scalar
                    eng.dma_start(
                        out=xt[:, s * 512:(s + 1) * 512],
                        in_=x_ext[ibt * BT + s, :].rearrange("(p f) -> p f", p=128),
                    )

                # ---- TT1 pass1: per sample, contract u ----
                w1 = [w1pool.tile([128, BT * 192], bf, tag=f"w1_{vc}", name=f"w1_{vc}")
                      for vc in range(2)]
                for pr in range(BT // 2):
                    for vc in range(2):
                        pw = ps_w.tile([128, 2 * 192], mybir.dt.float32, tag="pw")
                        for si in range(2):
                            s = pr * 2 + si
                            base = s * 512 + vc * 128
                            for c in range(2):
                                nc.tensor.matmul(
                                    pw[:, si * 192:(si + 1) * 192],
                                    lhsT=xt[:, base + c * 256:base + c * 256 + 128],
                                    rhs=sb["a1t"][:, c * 192:(c + 1) * 192],
                                    start=(c == 0),
                                    stop=(c == 1),
                                )
                        dst = w1[vc][:, pr * 384:(pr + 1) * 384]
                        if (pr * 2 + vc) % 3 == 1:
                            nc.scalar.activation(
                                dst, pw[:, :], mybir.ActivationFunctionType.Copy)
                        else:
                            nc.vector.tensor_copy(dst, pw[:, :])

                # ---- TT1 pass2: contract v and r, + bias, relu ----
                py = ps_y.tile([64, BT * I1], mybir.dt.float32, tag="py")
                k = 0
                for vc in range(2):
                    for r in range(R1):
                        nc.tensor.matmul(
                            py[:, :],
                            lhsT=sb["b1t"][:, vc * 192 + r * 64:vc * 192 + (r + 1) * 64],
                            rhs=w1[vc][:, :].rearrange("p (s m) -> p s m", m=192)
                                [:, :, r * 64:(r + 1) * 64],
                            start=(k == 0),
                            stop=False,
                        )
                        k += 1
                nc.tensor.matmul(
                    py[:, :], lhsT=sb["eye64"][:, :], rhs=sb["bias1rep"][:, :],
                    start=False, stop=True,
                )
                h1 = apool.tile([64, BT * I1], bf, tag="h1")   # [i2, (s,i1)] = [v2,(s,u2)]
                nc.scalar.activation(h1[:, :], py[:, :], mybir.ActivationFunctionType.Relu)

                # ---- TT2 pass1: per sample, contract v2 (input is flipped) ----
                pw2 = ps_s.tile([64, BT * 32], mybir.dt.float32, tag="s")
                for s in range(BT):
                    nc.tensor.matmul(
                        pw2[:, s * 32:(s + 1) * 32],
                        lhsT=h1[:, s * 64:(s + 1) * 64],
                        rhs=sb["b2stack"][:, :],
                        start=True, stop=True,
                    )
                w2 = apool.tile([64, BT * 32], bf, tag="w2")
                nc.vector.tensor_copy(w2[:, :], pw2[:, :])

                # ---- TT2 pass2: contract u2, r + bias, relu ----
                py2 = ps_s.tile([16, BT * J2], mybir.dt.float32, tag="s")
                for r in range(R2):
                    nc.tensor.matmul(
                        py2[:, :],
                        lhsT=sb["a2t"][:, r * 16:(r + 1) * 16],
                        rhs=w2[:, :].rearrange("p (s m) -> p s m", m=32)
                            [:, :, r * 16:(r + 1) * 16],
                        start=(r == 0), stop=False,
                    )
                nc.tensor.matmul(
                    py2[:, :], lhsT=sb["eye16"][:, :], rhs=sb["bias2rep"][:, :],
                    start=False, stop=True,
                )
                h2 = apool.tile([16, BT * J2], bf, tag="h2")
                nc.scalar.activation(h2[:, :], py2[:, :], mybir.ActivationFunctionType.Relu)

                # ---- TT3 pass1: per sample, contract u3 (input is normal) ----
                pw3 = ps_s.tile([16, BT * 16], mybir.dt.float32, tag="s")
                for s in range(BT):
                    nc.tensor.matmul(
                        pw3[:, s * 16:(s + 1) * 16],
                        lhsT=h2[:, s * 16:(s + 1) * 16],
                        rhs=sb["a3stack"][:, :],
                        start=True, stop=True,
                    )
                w3 = apool.tile([16, BT * 16], bf, tag="w3")
                nc.vector.tensor_copy(w3[:, :], pw3[:, :])

                # ---- TT3 pass2: contract v3, r + bias, relu ----
                py3 = ps_s.tile([8, BT * I3], mybir.dt.float32, tag="s")
                for r in range(R3):
                    nc.tensor.matmul(
                        py3[:, :],
                        lhsT=sb["b3t"][:, r * 8:(r + 1) * 8],
                        rhs=w3[:, :].rearrange("p (s m) -> p s m", m=16)
                            [:, :, r * 8:(r + 1) * 8],
                        start=(r == 0), stop=False,
                    )
                nc.tensor.matmul(
                    py3[:, :], lhsT=sb["eye8"][:, :], rhs=sb["bias3rep"][:, :],
                    start=False, stop=True,
                )
                nc.scalar.activation(
                    h3all[:, ibt * BT * I3:(ibt + 1) * BT * I3], py3[:, :],
                    mybir.ActivationFunctionType.Relu,
                )

            # ---- linear head over all 64 samples: contract (i1,i2) ----
            po = ps_y.tile([10, B_CORE], mybir.dt.float32, tag="py")
            for i1 in range(I3):
                nc.tensor.matmul(
                    po[:, :],
                    lhsT=sb["wlt"][:, i1 * 10:(i1 + 1) * 10],
                    rhs=h3all[:, :].rearrange("p (b m) -> p b m", m=I3)[:, :, i1],
                    start=(i1 == 0), stop=False,
                )
            nc.tensor.matmul(
                po[:, :], lhsT=sb["blin"][:, :], rhs=sb["ones64"][:, :],
                start=False, stop=True,
            )
            ysb = opool.tile([10, B_CORE], mybir.dt.float32)
            nc.vector.tensor_copy(ysb[:, :], po[:, :])
            nc.sync.dma_start(out=y_ext[:, :], in_=ysb[:, :])

    nc.finalize()
    return nc


def kernel(x, cores1, bias1, cores2, bias2, cores3, bias3, w_lin, b_lin, **extra):
    global _nc_cache
    x = np.ascontiguousarray(
        np.asarray(x, dtype=np.float32).reshape(BATCH, FEAT).astype(BF16))
    w = _prep_weights(cores1, bias1, cores2, bias2, cores3, bias3, w_lin, b_lin)

    if _nc_cache is None:
        _nc_cache = _build_nc()
    nc = _nc_cache

    in_maps = []
    for i in range(N_CORES):
        m = {"x": x[i * B_CORE:(i + 1) * B_CORE]}
        m.update(w)
        in_maps.append(m)

    import os
    trace = bool(int(os.environ.get("KERNEL_TRACE", "0")))
    res = run_bass_kernel_spmd(
        nc, in_maps, core_ids=list(range(N_CORES)), trace=trace,
        trace_cores=[0] if trace else None,
    )
    global last_results
    last_results = res
    outs = [res.results[i]["y"].T for i in range(N_CORES)]   # [64, 10] each
    return np.concatenate(outs, axis=0).astype(np.float32)


last_results = None


# revision 24
# speedup vs baseline: 1.0238x; 1.0238x over previous
"""Trainium2 Bass kernel for nn_BasicTT: 3 TT layers + linear head.

Strategy
--------
Data-parallel over batch: 512 samples -> 8 cores x 64 samples. Weights
replicated. Each TT layer (cores c1..c6, mid rank R) is evaluated via the
exact Kronecker 3+3 merge

    y[(i1,i2)] = sum_r A_r @ X @ B_r^T,   A[i1,u,R], B[R,i2,v], X=[u,v]

which is two TensorE passes per layer and needs no on-chip transposes:
 pass1 (per sample): stationary lhsT = X chunk [u(K), v(M)], moving
   rhs = A^T stacked [u, (r,i1)]  ->  W[v, (r,i1)]      (PE contracts u)
 pass2 (per batch-tile): stationary lhsT = B^T [v, i2], moving
   rhs = W [v, (s,i1)] per r, accumulated in PSUM       (PE contracts v,r)
Layer biases are folded in as one extra identity matmul into the same PSUM
accumulation; ReLU is fused into the PSUM->SBUF copy on ScalarE.
Layer outputs alternate orientation ([i2, i1] vs [i1, i2]) which exactly
matches the next layer's expected (v-on-partitions / u-on-partitions) input,
so the whole network chains with zero data reshuffling.

Compute dtype is bf16 (f32 PSUM accumulation); verified end-to-end error vs
the f32 reference is ~6e-3 relative.
"""

import sys

sys.path.insert(0, "/opt/trn_rl_repo")

import numpy as np
import ml_dtypes

import concourse.bass as bass
import concourse.bacc as bacc
import concourse.mybir as mybir
from concourse.tile import TileContext
from concourse.bass_utils import run_bass_kernel_spmd

BF16 = ml_dtypes.bfloat16
N_CORES = 8
BATCH = 512
B_CORE = BATCH // N_CORES          # 64 samples per core
BT = 8                             # batch sub-tile (samples per inner iteration)
N_BT = B_CORE // BT

FEAT = 65536                       # 4*8*8*4*8*8
U1, V1, R1, I1, J1 = 256, 256, 3, 64, 64     # TT1: u,v,midrank, m_first, m_last
U2, V2, R2, I2, J2 = 64, 64, 2, 16, 16       # TT2
U3, V3, R3, I3, J3 = 16, 16, 2, 8, 8         # TT3


def _merge_first3(cores):
    """cores[0..2] -> A[i1, u, R] (m-major / n-major flattening)."""
    c1, c2, c3 = [np.asarray(c, np.float32) for c in cores]
    A = np.einsum("amnr,rpqs,sxyt->mpxnqyt", c1, c2, c3)
    return A.reshape(
        c1.shape[1] * c2.shape[1] * c3.shape[1],
        c1.shape[2] * c2.shape[2] * c3.shape[2],
        c3.shape[3],
    )


def _merge_last3(cores):
    """cores[3..5] -> B[R, i2, v]."""
    c4, c5, c6 = [np.asarray(c, np.float32) for c in cores]
    B = np.einsum("amnr,rpqs,sxyt->ampxnqy", c4, c5, c6)
    return B.reshape(
        c4.shape[0],
        c4.shape[1] * c5.shape[1] * c6.shape[1],
        c4.shape[2] * c5.shape[2] * c6.shape[2],
    )


def _prep_weights(cores1, bias1, cores2, bias2, cores3, bias3, w_lin, b_lin):
    """Host-side merge + layout. All outputs bf16 (except noted)."""
    w = {}

    # ---- TT1 ----
    A1 = _merge_first3(cores1[:3])            # [64, 256, 3]
    B1 = _merge_last3(cores1[3:])             # [3, 64, 256]
    # pass1 moving operand: a1t[c][p, r*64+i1] = A1[i1, u=2p+c, r]
    a1t = A1.transpose(1, 2, 0).reshape(U1, R1 * I1)     # [u, (r,i1)]
    a1t = np.stack([a1t[0::2, :], a1t[1::2, :]])         # [2, 128, 192]
    w["a1t"] = a1t.astype(BF16)
    # pass2 stationary: b1t[vc][v_lo, r*64+i2] = B1[r, i2, v=vc*128+v_lo]
    b1t = B1.transpose(2, 0, 1).reshape(V1, R1 * J1)     # [v, (r,i2)]
    w["b1t"] = np.stack([b1t[:128], b1t[128:]]).astype(BF16)  # [2, 128, 192]
    # bias1 in output layout [i2, i1], replicated BT times along free-major s
    b1 = np.asarray(bias1, np.float32).reshape(I1, J1).T  # [i2, i1]
    w["bias1rep"] = np.tile(b1[:, None, :], (1, BT, 1)).reshape(J1, BT * I1).astype(BF16)

    # ---- TT2 (input arrives flipped: [v2, u2] on partitions) ----
    A2 = _merge_first3(cores2[:3])            # [16, 64, 2]
    B2 = _merge_last3(cores2[3:])             # [2, 16, 64]
    # pass1 moving: b2stack[v2, (r,j2)] = B2[r, j2, v2]
    w["b2stack"] = B2.transpose(2, 0, 1).reshape(V2, R2 * J2).astype(BF16)  # [64, 32]
    # pass2 stationary: a2t[r][u2, j1] = A2[j1, u2, r]
    w["a2t"] = A2.transpose(2, 1, 0).astype(BF16)        # [2, 64, 16]
    b2 = np.asarray(bias2, np.float32).reshape(16, 16)   # [j1, j2] j1-major
    w["bias2rep"] = np.tile(b2[:, None, :], (1, BT, 1)).reshape(16, BT * 16).astype(BF16)

    # ---- TT3 (input arrives normal: [u3, v3-ish free]) ----
    A3 = _merge_first3(cores3[:3])            # [8, 16, 2]
    B3 = _merge_last3(cores3[3:])             # [2, 8, 16]
    # pass1 moving: a3stack[u3, (r,i1)] = A3[i1, u3, r]
    w["a3stack"] = A3.transpose(1, 2, 0).reshape(U3, R3 * I3).astype(BF16)  # [16, 16]
    # pass2 stationary: b3t[r][v3, i2] = B3[r, i2, v3]
    w["b3t"] = B3.transpose(2, 0, 1).astype(BF16)        # [16, 2, 8] -> index [v3, r, i2]
    b3 = np.asarray(bias3, np.float32).reshape(I3 * J3)  # flat (i1,i2) i1-major
    b3 = b3.reshape(I3, J3).T                            # [i2, i1]
    w["bias3rep"] = np.tile(b3[:, None, :], (1, BT, 1)).reshape(J3, BT * I3).astype(BF16)

    # ---- linear head ----
    wl = np.asarray(w_lin, np.float32).reshape(10, I3, J3)  # [o, i1, i2]
    w["wlt"] = wl.transpose(2, 1, 0).astype(BF16)        # [i2=8, i1=8, 10]
    w["blin"] = np.asarray(b_lin, np.float32).reshape(1, 10).astype(BF16)

    # identities for the bias matmuls, ones row for the head bias
    w["eye64"] = np.eye(64, dtype=np.float32).astype(BF16)
    w["eye16"] = np.eye(16, dtype=np.float32).astype(BF16)
    w["eye8"] = np.eye(8, dtype=np.float32).astype(BF16)
    w["ones64"] = np.ones((1, 64), dtype=np.float32).astype(BF16)

    # pack everything into a single [128, WPACK_COLS] blob: one DMA at start
    pack = np.zeros((128, WPACK_COLS), dtype=BF16)
    def put2d(arr2d, name):
        rows, cols, off = WPACK_SLOTS[name]
        assert arr2d.shape == (rows, cols), (name, arr2d.shape)
        pack[:rows, off:off + cols] = arr2d
    put2d(np.concatenate([w["a1t"][0], w["a1t"][1]], axis=1), "a1t")
    put2d(np.concatenate([w["b1t"][0], w["b1t"][1]], axis=1), "b1t")
    put2d(w["bias1rep"], "bias1rep")
    put2d(w["b2stack"], "b2stack")
    a2t2d = np.concatenate([w["a2t"][0], w["a2t"][1]], axis=1)
    put2d(a2t2d, "a2t")
    pack[64:128, 1312:1344] = a2t2d          # copy for the odd-half row group
    put2d(w["bias2rep"], "bias2rep")
    put2d(w["a3stack"], "a3stack")
    b3t2d = w["b3t"].reshape(16, 16)
    put2d(b3t2d, "b3t")
    for k in (1, 2, 3):                      # copies for row groups 32/64/96
        pack[32 * k:32 * k + 16, 1488:1504] = b3t2d
    put2d(w["bias3rep"], "bias3rep")
    put2d(w["wlt"].reshape(8, 80), "wlt")
    put2d(w["blin"], "blin")
    put2d(w["eye64"], "eye64")
    put2d(w["eye16"], "eye16")
    put2d(w["eye8"], "eye8")
    put2d(w["ones64"], "ones64")
    return {"wpack": pack}


# name -> (rows, cols, col_offset) in the packed weight blob
WPACK_SLOTS = {
    "a1t": (128, 384, 0), "b1t": (128, 384, 384), "bias1rep": (64, 512, 768),
    "b2stack": (64, 32, 1280), "a2t": (64, 32, 1312), "bias2rep": (16, 128, 1344),
    "a3stack": (16, 16, 1472), "b3t": (16, 16, 1488), "bias3rep": (8, 64, 1504),
    "wlt": (8, 80, 1568), "blin": (1, 10, 1648), "eye64": (64, 64, 1664),
    "eye16": (16, 16, 1728), "eye8": (8, 8, 1744), "ones64": (1, 64, 1760),
}
WPACK_COLS = 1824

_nc_cache = None


def _build_nc():
    f32 = mybir.dt.float32
    bf = mybir.dt.bfloat16
    nc = bacc.Bacc()

    x_ext = nc.declare_dram_parameter("x", [B_CORE, FEAT], bf, isOutput=False)
    wp_ext = nc.declare_dram_parameter("wpack", [128, WPACK_COLS], bf, isOutput=False)
    y_ext = nc.declare_dram_parameter("y", [10, B_CORE], f32, isOutput=True)

    with TileContext(nc) as tc:
        with (
            tc.tile_pool(name="consts", bufs=1) as cpool,
            tc.tile_pool(name="xt", bufs=3) as xpool,
            tc.tile_pool(name="w1", bufs=2) as w1pool,
            tc.tile_pool(name="act", bufs=2) as apool,
            tc.tile_pool(name="h3", bufs=1) as h3pool,
            tc.tile_pool(name="out", bufs=1) as opool,
            tc.tile_pool(name="ps_w", bufs=4, space="PSUM") as ps_w,
            tc.tile_pool(name="ps_y", bufs=2, space="PSUM") as ps_y,
            tc.tile_pool(name="ps_s", bufs=2, space="PSUM") as ps_s,
        ):
            # -- load all weights with a single DMA on the SWDGE path, so
            # -- it never queues behind the HWDGE x loads --
            wsb = cpool.tile([128, WPACK_COLS], bf)
            # a1t first: TT1 pass1 only needs cols 0:384, so the first matmul
            # can start as soon as this small DMA lands
            nc.gpsimd.dma_start(out=wsb[:, :384], in_=wp_ext[:, :384])
            nc.gpsimd.dma_start(out=wsb[:, 384:], in_=wp_ext[:, 384:])
            sb = {
                name: wsb[:rows, off:off + cols]
                for name, (rows, cols, off) in WPACK_SLOTS.items()
            }

            h3all = h3pool.tile([8, B_CORE * I3], bf)   # [i2'', (b, i1'')]

            for ibt in range(N_BT):
                # ---- load BT samples, cast f32 -> bf16 on the fly ----
                xt = xpool.tile([128, BT * 512], bf)     # free = (s, u_lo=2, v=256)
                # per-sample DMAs so pass1 for sample s starts as soon as its
                # 128KB lands instead of waiting for the full 1MB tile
                for s in range(BT):
                    nc.sync.dma_start(
                        out=xt[:, s * 512:(s + 1) * 512],
                        in_=x_ext[ibt * BT + s, :].rearrange("(p f) -> p f", p=128),
                    )

                # ---- TT1 pass1: per sample, contract u ----
                w1 = [w1pool.tile([128, BT * 192], bf, tag=f"w1_{vc}", name=f"w1_{vc}")
                      for vc in range(2)]
                for pr in range(BT // 2):
                    for vc in range(2):
                        pw = ps_w.tile([128, 2 * 192], mybir.dt.float32, tag="pw")
                        for si in range(2):
                            s = pr * 2 + si
                            base = s * 512 + vc * 128
                            for c in range(2):
                                nc.tensor.matmul(
                                    pw[:, si * 192:(si + 1) * 192],
                                    lhsT=xt[:, base + c * 256:base + c * 256 + 128],
                                    rhs=sb["a1t"][:, c * 192:(c + 1) * 192],
                                    start=(c == 0),
                                    stop=(c == 1),
                                )
                        dst = w1[vc][:, pr * 384:(pr + 1) * 384]
                        if (pr * 2 + vc) % 3 == 1:
                            nc.scalar.activation(
                                dst, pw[:, :], mybir.ActivationFunctionType.Copy)
                        else:
                            nc.vector.tensor_copy(dst, pw[:, :])

                # ---- TT1 pass2: contract v and r, + bias, relu ----
                py = ps_y.tile([64, BT * I1], mybir.dt.float32, tag="py")
                k = 0
                for vc in range(2):
                    for r in range(R1):
                        nc.tensor.matmul(
                            py[:, :],
                            lhsT=sb["b1t"][:, vc * 192 + r * 64:vc * 192 + (r + 1) * 64],
                            rhs=w1[vc][:, :].rearrange("p (s m) -> p s m", m=192)
                                [:, :, r * 64:(r + 1) * 64],
                            start=(k == 0),
                            stop=False,
                        )
                        k += 1
                nc.tensor.matmul(
                    py[:, :], lhsT=sb["eye64"][:, :], rhs=sb["bias1rep"][:, :],
                    start=False, stop=True,
                )
                h1 = apool.tile([64, BT * I1], bf, tag="h1")   # [i2, (s,i1)] = [v2,(s,u2)]
                nc.scalar.activation(h1[:, :], py[:, :], mybir.ActivationFunctionType.Relu)

                # ---- TT2 pass1: per sample, contract v2 (input is flipped) ----
                pw2 = ps_s.tile([64, BT * 32], mybir.dt.float32, tag="s")
                for s in range(BT):
                    nc.tensor.matmul(
                        pw2[:, s * 32:(s + 1) * 32],
                        lhsT=h1[:, s * 64:(s + 1) * 64],
                        rhs=sb["b2stack"][:, :],
                        start=True, stop=True,
                    )
                w2 = apool.tile([64, BT * 32], bf, tag="w2")
                nc.vector.tensor_copy(w2[:, :], pw2[:, :])

                # ---- TT2 pass2: contract u2, r + bias, relu ----
                py2 = ps_s.tile([16, BT * J2], mybir.dt.float32, tag="s")
                for r in range(R2):
                    nc.tensor.matmul(
                        py2[:, :],
                        lhsT=sb["a2t"][:, r * 16:(r + 1) * 16],
                        rhs=w2[:, :].rearrange("p (s m) -> p s m", m=32)
                            [:, :, r * 16:(r + 1) * 16],
                        start=(r == 0), stop=False,
                    )
                nc.tensor.matmul(
                    py2[:, :], lhsT=sb["eye16"][:, :], rhs=sb["bias2rep"][:, :],
                    start=False, stop=True,
                )
                h2 = apool.tile([16, BT * J2], bf, tag="h2")
                nc.scalar.activation(h2[:, :], py2[:, :], mybir.ActivationFunctionType.Relu)

                # ---- TT3 pass1: per sample, contract u3 (input is normal) ----
                pw3 = ps_s.tile([16, BT * 16], mybir.dt.float32, tag="s")
                for s in range(BT):
                    nc.tensor.matmul(
                        pw3[:, s * 16:(s + 1) * 16],
                        lhsT=h2[:, s * 16:(s + 1) * 16],
                        rhs=sb["a3stack"][:, :],
                        start=True, stop=True,
                    )
                w3 = apool.tile([16, BT * 16], bf, tag="w3")
                nc.vector.tensor_copy(w3[:, :], pw3[:, :])

                # ---- TT3 pass2: contract v3, r + bias, relu ----
                py3 = ps_s.tile([8, BT * I3], mybir.dt.float32, tag="s")
                for r in range(R3):
                    nc.tensor.matmul(
                        py3[:, :],
                        lhsT=sb["b3t"][:, r * 8:(r + 1) * 8],
                        rhs=w3[:, :].rearrange("p (s m) -> p s m", m=16)
                            [:, :, r * 8:(r + 1) * 8],
                        start=(r == 0), stop=False,
                    )
                nc.tensor.matmul(
                    py3[:, :], lhsT=sb["eye8"][:, :], rhs=sb["bias3rep"][:, :],
                    start=False, stop=True,
                )
                nc.scalar.activation(
                    h3all[:, ibt * BT * I3:(ibt + 1) * BT * I3], py3[:, :],
                    mybir.ActivationFunctionType.Relu,
                )

            # ---- linear head over all 64 samples: contract (i1,i2) ----
            po = ps_y.tile([10, B_CORE], mybir.dt.float32, tag="py")
            for i1 in range(I3):
                nc.tensor.matmul(
                    po[:, :],
                    lhsT=sb["wlt"][:, i1 * 10:(i1 + 1) * 10],
                    rhs=h3all[:, :].rearrange("p (b m) -> p b m", m=I3)[:, :, i1],
                    start=(i1 == 0), stop=False,
                )
            nc.tensor.matmul(
                po[:, :], lhsT=sb["blin"][:, :], rhs=sb["ones64"][:, :],
                start=False, stop=True,
            )
            ysb = opool.tile([10, B_CORE], mybir.dt.float32)
            nc.vector.tensor_copy(ysb[:, :], po[:, :])
            nc.sync.dma_start(out=y_ext[:, :], in_=ysb[:, :])

    nc.finalize()
    return nc


def kernel(x, cores1, bias1, cores2, bias2, cores3, bias3, w_lin, b_lin, **extra):
    global _nc_cache
    x = np.ascontiguousarray(
        np.asarray(x, dtype=np.float32).reshape(BATCH, FEAT).astype(BF16))
    w = _prep_weights(cores1, bias1, cores2, bias2, cores3, bias3, w_lin, b_lin)

    if _nc_cache is None:
        _nc_cache = _build_nc()
    nc = _nc_cache

    in_maps = []
    for i in range(N_CORES):
        m = {"x": x[i * B_CORE:(i + 1) * B_CORE]}
        m.update(w)
        in_maps.append(m)

    import os
    trace = bool(int(os.environ.get("KERNEL_TRACE", "0")))
    res = run_bass_kernel_spmd(
        nc, in_maps, core_ids=list(range(N_CORES)), trace=trace,
        trace_cores=[0] if trace else None,
    )
    global last_results
    last_results = res
    outs = [res.results[i]["y"].T for i in range(N_CORES)]   # [64, 10] each
    return np.concatenate(outs, axis=0).astype(np.float32)


last_results = None


# revision 25
# speedup vs baseline: 1.0315x; 1.0076x over previous
"""Trainium2 Bass kernel for nn_BasicTT: 3 TT layers + linear head.

Strategy
--------
Data-parallel over batch: 512 samples -> 8 cores x 64 samples. Weights
replicated. Each TT layer (cores c1..c6, mid rank R) is evaluated via the
exact Kronecker 3+3 merge

    y[(i1,i2)] = sum_r A_r @ X @ B_r^T,   A[i1,u,R], B[R,i2,v], X=[u,v]

which is two TensorE passes per layer and needs no on-chip transposes:
 pass1 (per sample): stationary lhsT = X chunk [u(K), v(M)], moving
   rhs = A^T stacked [u, (r,i1)]  ->  W[v, (r,i1)]      (PE contracts u)
 pass2 (per batch-tile): stationary lhsT = B^T [v, i2], moving
   rhs = W [v, (s,i1)] per r, accumulated in PSUM       (PE contracts v,r)
Layer biases are folded in as one extra identity matmul into the same PSUM
accumulation; ReLU is fused into the PSUM->SBUF copy on ScalarE.
Layer outputs alternate orientation ([i2, i1] vs [i1, i2]) which exactly
matches the next layer's expected (v-on-partitions / u-on-partitions) input,
so the whole network chains with zero data reshuffling.

Compute dtype is bf16 (f32 PSUM accumulation); verified end-to-end error vs
the f32 reference is ~6e-3 relative.
"""

import sys

sys.path.insert(0, "/opt/trn_rl_repo")

import numpy as np
import ml_dtypes

import concourse.bass as bass
import concourse.bacc as bacc
import concourse.mybir as mybir
from concourse.tile import TileContext
from concourse.bass_utils import run_bass_kernel_spmd

BF16 = ml_dtypes.bfloat16
N_CORES = 8
BATCH = 512
B_CORE = BATCH // N_CORES          # 64 samples per core
BT = 8                             # batch sub-tile (samples per inner iteration)
N_BT = B_CORE // BT

FEAT = 65536                       # 4*8*8*4*8*8
U1, V1, R1, I1, J1 = 256, 256, 3, 64, 64     # TT1: u,v,midrank, m_first, m_last
U2, V2, R2, I2, J2 = 64, 64, 2, 16, 16       # TT2
U3, V3, R3, I3, J3 = 16, 16, 2, 8, 8         # TT3


def _merge_first3(cores):
    """cores[0..2] -> A[i1, u, R] (m-major / n-major flattening)."""
    c1, c2, c3 = [np.asarray(c, np.float32) for c in cores]
    A = np.einsum("amnr,rpqs,sxyt->mpxnqyt", c1, c2, c3)
    return A.reshape(
        c1.shape[1] * c2.shape[1] * c3.shape[1],
        c1.shape[2] * c2.shape[2] * c3.shape[2],
        c3.shape[3],
    )


def _merge_last3(cores):
    """cores[3..5] -> B[R, i2, v]."""
    c4, c5, c6 = [np.asarray(c, np.float32) for c in cores]
    B = np.einsum("amnr,rpqs,sxyt->ampxnqy", c4, c5, c6)
    return B.reshape(
        c4.shape[0],
        c4.shape[1] * c5.shape[1] * c6.shape[1],
        c4.shape[2] * c5.shape[2] * c6.shape[2],
    )


def _prep_weights(cores1, bias1, cores2, bias2, cores3, bias3, w_lin, b_lin):
    """Host-side merge + layout. All outputs bf16 (except noted)."""
    w = {}

    # ---- TT1 ----
    A1 = _merge_first3(cores1[:3])            # [64, 256, 3]
    B1 = _merge_last3(cores1[3:])             # [3, 64, 256]
    # pass1 moving operand: a1t[c][p, r*64+i1] = A1[i1, u=2p+c, r]
    a1t = A1.transpose(1, 2, 0).reshape(U1, R1 * I1)     # [u, (r,i1)]
    a1t = np.stack([a1t[0::2, :], a1t[1::2, :]])         # [2, 128, 192]
    w["a1t"] = a1t.astype(BF16)
    # pass2 stationary: b1t[vc][v_lo, r*64+i2] = B1[r, i2, v=vc*128+v_lo]
    b1t = B1.transpose(2, 0, 1).reshape(V1, R1 * J1)     # [v, (r,i2)]
    w["b1t"] = np.stack([b1t[:128], b1t[128:]]).astype(BF16)  # [2, 128, 192]
    # bias1 in output layout [i2, i1], replicated BT times along free-major s
    b1 = np.asarray(bias1, np.float32).reshape(I1, J1).T  # [i2, i1]
    w["bias1rep"] = np.tile(b1[:, None, :], (1, BT, 1)).reshape(J1, BT * I1).astype(BF16)

    # ---- TT2 (input arrives flipped: [v2, u2] on partitions) ----
    A2 = _merge_first3(cores2[:3])            # [16, 64, 2]
    B2 = _merge_last3(cores2[3:])             # [2, 16, 64]
    # pass1 moving: b2stack[v2, (r,j2)] = B2[r, j2, v2]
    w["b2stack"] = B2.transpose(2, 0, 1).reshape(V2, R2 * J2).astype(BF16)  # [64, 32]
    # pass2 stationary: a2t[r][u2, j1] = A2[j1, u2, r]
    w["a2t"] = A2.transpose(2, 1, 0).astype(BF16)        # [2, 64, 16]
    b2 = np.asarray(bias2, np.float32).reshape(16, 16)   # [j1, j2] j1-major
    w["bias2rep"] = np.tile(b2[:, None, :], (1, BT, 1)).reshape(16, BT * 16).astype(BF16)

    # ---- TT3 (input arrives normal: [u3, v3-ish free]) ----
    A3 = _merge_first3(cores3[:3])            # [8, 16, 2]
    B3 = _merge_last3(cores3[3:])             # [2, 8, 16]
    # pass1 moving: a3stack[u3, (r,i1)] = A3[i1, u3, r]
    w["a3stack"] = A3.transpose(1, 2, 0).reshape(U3, R3 * I3).astype(BF16)  # [16, 16]
    # pass2 stationary: b3t[r][v3, i2] = B3[r, i2, v3]
    w["b3t"] = B3.transpose(2, 0, 1).astype(BF16)        # [16, 2, 8] -> index [v3, r, i2]
    b3 = np.asarray(bias3, np.float32).reshape(I3 * J3)  # flat (i1,i2) i1-major
    b3 = b3.reshape(I3, J3).T                            # [i2, i1]
    w["bias3rep"] = np.tile(b3[:, None, :], (1, BT, 1)).reshape(J3, BT * I3).astype(BF16)

    # ---- linear head ----
    wl = np.asarray(w_lin, np.float32).reshape(10, I3, J3)  # [o, i1, i2]
    w["wlt"] = wl.transpose(2, 1, 0).astype(BF16)        # [i2=8, i1=8, 10]
    w["blin"] = np.asarray(b_lin, np.float32).reshape(1, 10).astype(BF16)

    # identities for the bias matmuls, ones row for the head bias
    w["eye64"] = np.eye(64, dtype=np.float32).astype(BF16)
    w["eye16"] = np.eye(16, dtype=np.float32).astype(BF16)
    w["eye8"] = np.eye(8, dtype=np.float32).astype(BF16)
    w["ones64"] = np.ones((1, 64), dtype=np.float32).astype(BF16)

    # pack everything into a single [128, WPACK_COLS] blob: one DMA at start
    pack = np.zeros((128, WPACK_COLS), dtype=BF16)
    def put2d(arr2d, name):
        rows, cols, off = WPACK_SLOTS[name]
        assert arr2d.shape == (rows, cols), (name, arr2d.shape)
        pack[:rows, off:off + cols] = arr2d
    put2d(np.concatenate([w["a1t"][0], w["a1t"][1]], axis=1), "a1t")
    put2d(np.concatenate([w["b1t"][0], w["b1t"][1]], axis=1), "b1t")
    put2d(w["bias1rep"], "bias1rep")
    put2d(w["b2stack"], "b2stack")
    a2t2d = np.concatenate([w["a2t"][0], w["a2t"][1]], axis=1)
    put2d(a2t2d, "a2t")
    pack[64:128, 1312:1344] = a2t2d          # copy for the odd-half row group
    put2d(w["bias2rep"], "bias2rep")
    put2d(w["a3stack"], "a3stack")
    b3t2d = w["b3t"].reshape(16, 16)
    put2d(b3t2d, "b3t")
    for k in (1, 2, 3):                      # copies for row groups 32/64/96
        pack[32 * k:32 * k + 16, 1488:1504] = b3t2d
    put2d(w["bias3rep"], "bias3rep")
    put2d(w["wlt"].reshape(8, 80), "wlt")
    put2d(w["blin"], "blin")
    put2d(w["eye64"], "eye64")
    put2d(w["eye16"], "eye16")
    put2d(w["eye8"], "eye8")
    put2d(w["ones64"], "ones64")
    return {"wpack": pack}


# name -> (rows, cols, col_offset) in the packed weight blob
WPACK_SLOTS = {
    "a1t": (128, 384, 0), "b1t": (128, 384, 384), "bias1rep": (64, 512, 768),
    "b2stack": (64, 32, 1280), "a2t": (64, 32, 1312), "bias2rep": (16, 128, 1344),
    "a3stack": (16, 16, 1472), "b3t": (16, 16, 1488), "bias3rep": (8, 64, 1504),
    "wlt": (8, 80, 1568), "blin": (1, 10, 1648), "eye64": (64, 64, 1664),
    "eye16": (16, 16, 1728), "eye8": (8, 8, 1744), "ones64": (1, 64, 1760),
}
WPACK_COLS = 1824

_nc_cache = None


def _build_nc():
    f32 = mybir.dt.float32
    bf = mybir.dt.bfloat16
    nc = bacc.Bacc()

    x_ext = nc.declare_dram_parameter("x", [B_CORE, FEAT], bf, isOutput=False)
    wp_ext = nc.declare_dram_parameter("wpack", [128, WPACK_COLS], bf, isOutput=False)
    y_ext = nc.declare_dram_parameter("y", [10, B_CORE], f32, isOutput=True)

    with TileContext(nc) as tc:
        with (
            tc.tile_pool(name="consts", bufs=1) as cpool,
            tc.tile_pool(name="xt", bufs=3) as xpool,
            tc.tile_pool(name="w1", bufs=2) as w1pool,
            tc.tile_pool(name="act", bufs=2) as apool,
            tc.tile_pool(name="h3", bufs=1) as h3pool,
            tc.tile_pool(name="out", bufs=1) as opool,
            tc.tile_pool(name="ps_w", bufs=4, space="PSUM") as ps_w,
            tc.tile_pool(name="ps_y", bufs=2, space="PSUM") as ps_y,
            tc.tile_pool(name="ps_s", bufs=2, space="PSUM") as ps_s,
        ):
            # -- load all weights with a single DMA on the SWDGE path, so
            # -- it never queues behind the HWDGE x loads --
            wsb = cpool.tile([128, WPACK_COLS], bf)
            # a1t first: TT1 pass1 only needs cols 0:384, so the first matmul
            # can start as soon as this small DMA lands
            nc.gpsimd.dma_start(out=wsb[:, :384], in_=wp_ext[:, :384])
            nc.gpsimd.dma_start(out=wsb[:, 384:], in_=wp_ext[:, 384:])
            sb = {
                name: wsb[:rows, off:off + cols]
                for name, (rows, cols, off) in WPACK_SLOTS.items()
            }

            h3all = h3pool.tile([8, B_CORE * I3], bf)   # [i2'', (b, i1'')]

            for ibt in range(N_BT):
                # ---- load BT samples, cast f32 -> bf16 on the fly ----
                xt = xpool.tile([128, BT * 512], bf)     # free = (s, u_lo=2, v=256)
                # per-sample DMAs so pass1 for sample s starts as soon as its
                # 128KB lands instead of waiting for the full 1MB tile
                for s in range(BT):
                    nc.sync.dma_start(
                        out=xt[:, s * 512:(s + 1) * 512],
                        in_=x_ext[ibt * BT + s, :].rearrange("(p f) -> p f", p=128),
                    )

                # ---- TT1 pass1: per sample, contract u ----
                w1 = [w1pool.tile([128, BT * 192], bf, tag=f"w1_{vc}", name=f"w1_{vc}")
                      for vc in range(2)]
                for pr in range(BT // 2):
                    for vc in range(2):
                        pw = ps_w.tile([128, 2 * 192], mybir.dt.float32, tag="pw")
                        for si in range(2):
                            s = pr * 2 + si
                            base = s * 512 + vc * 128
                            for c in range(2):
                                nc.tensor.matmul(
                                    pw[:, si * 192:(si + 1) * 192],
                                    lhsT=xt[:, base + c * 256:base + c * 256 + 128],
                                    rhs=sb["a1t"][:, c * 192:(c + 1) * 192],
                                    start=(c == 0),
                                    stop=(c == 1),
                                )
                        dst = w1[vc][:, pr * 384:(pr + 1) * 384]
                        if (pr * 2 + vc) % 3 == 1:
                            nc.scalar.activation(
                                dst, pw[:, :], mybir.ActivationFunctionType.Copy)
                        else:
                            nc.vector.tensor_copy(dst, pw[:, :])

                # ---- TT1 pass2: contract v and r, + bias, relu ----
                py = ps_y.tile([64, BT * I1], mybir.dt.float32, tag="py")
                k = 0
                for vc in range(2):
                    for r in range(R1):
                        nc.tensor.matmul(
                            py[:, :],
                            lhsT=sb["b1t"][:, vc * 192 + r * 64:vc * 192 + (r + 1) * 64],
                            rhs=w1[vc][:, :].rearrange("p (s m) -> p s m", m=192)
                                [:, :, r * 64:(r + 1) * 64],
                            start=(k == 0),
                            stop=(k == 2 * R1 - 1),
                        )
                        k += 1
                nc.vector.tensor_add(py[:, :], py[:, :], sb["bias1rep"][:, :])
                h1 = apool.tile([64, BT * I1], bf, tag="h1")   # [i2, (s,i1)] = [v2,(s,u2)]
                nc.scalar.activation(h1[:, :], py[:, :], mybir.ActivationFunctionType.Relu)

                # ---- TT2 pass1: per sample, contract v2 (input is flipped) ----
                pw2 = ps_s.tile([64, BT * 32], mybir.dt.float32, tag="s")
                for s in range(BT):
                    nc.tensor.matmul(
                        pw2[:, s * 32:(s + 1) * 32],
                        lhsT=h1[:, s * 64:(s + 1) * 64],
                        rhs=sb["b2stack"][:, :],
                        start=True, stop=True,
                    )
                w2 = apool.tile([64, BT * 32], bf, tag="w2")
                nc.vector.tensor_copy(w2[:, :], pw2[:, :])

                # ---- TT2 pass2: contract u2, r + bias, relu ----
                py2 = ps_s.tile([16, BT * J2], mybir.dt.float32, tag="s")
                for r in range(R2):
                    nc.tensor.matmul(
                        py2[:, :],
                        lhsT=sb["a2t"][:, r * 16:(r + 1) * 16],
                        rhs=w2[:, :].rearrange("p (s m) -> p s m", m=32)
                            [:, :, r * 16:(r + 1) * 16],
                        start=(r == 0), stop=(r == R2 - 1),
                    )
                nc.vector.tensor_add(py2[:, :], py2[:, :], sb["bias2rep"][:, :])
                h2 = apool.tile([16, BT * J2], bf, tag="h2")
                nc.scalar.activation(h2[:, :], py2[:, :], mybir.ActivationFunctionType.Relu)

                # ---- TT3 pass1: per sample, contract u3 (input is normal) ----
                pw3 = ps_s.tile([16, BT * 16], mybir.dt.float32, tag="s")
                for s in range(BT):
                    nc.tensor.matmul(
                        pw3[:, s * 16:(s + 1) * 16],
                        lhsT=h2[:, s * 16:(s + 1) * 16],
                        rhs=sb["a3stack"][:, :],
                        start=True, stop=True,
                    )
                w3 = apool.tile([16, BT * 16], bf, tag="w3")
                nc.vector.tensor_copy(w3[:, :], pw3[:, :])

                # ---- TT3 pass2: contract v3, r + bias, relu ----
                py3 = ps_s.tile([8, BT * I3], mybir.dt.float32, tag="s")
                for r in range(R3):
                    nc.tensor.matmul(
                        py3[:, :],
                        lhsT=sb["b3t"][:, r * 8:(r + 1) * 8],
                        rhs=w3[:, :].rearrange("p (s m) -> p s m", m=16)
                            [:, :, r * 8:(r + 1) * 8],
                        start=(r == 0), stop=(r == R3 - 1),
                    )
                nc.vector.tensor_add(py3[:, :], py3[:, :], sb["bias3rep"][:, :])
                nc.scalar.activation(
                    h3all[:, ibt * BT * I3:(ibt + 1) * BT * I3], py3[:, :],
                    mybir.ActivationFunctionType.Relu,
                )

            # ---- linear head over all 64 samples: contract (i1,i2) ----
            po = ps_y.tile([10, B_CORE], mybir.dt.float32, tag="py")
            for i1 in range(I3):
                nc.tensor.matmul(
                    po[:, :],
                    lhsT=sb["wlt"][:, i1 * 10:(i1 + 1) * 10],
                    rhs=h3all[:, :].rearrange("p (b m) -> p b m", m=I3)[:, :, i1],
                    start=(i1 == 0), stop=False,
                )
            nc.tensor.matmul(
                po[:, :], lhsT=sb["blin"][:, :], rhs=sb["ones64"][:, :],
                start=False, stop=True,
            )
            ysb = opool.tile([10, B_CORE], mybir.dt.float32)
            nc.vector.tensor_copy(ysb[:, :], po[:, :])
            nc.sync.dma_start(out=y_ext[:, :], in_=ysb[:, :])

    nc.finalize()
    return nc


def kernel(x, cores1, bias1, cores2, bias2, cores3, bias3, w_lin, b_lin, **extra):
    global _nc_cache
    x = np.ascontiguousarray(
        np.asarray(x, dtype=np.float32).reshape(BATCH, FEAT).astype(BF16))
    w = _prep_weights(cores1, bias1, cores2, bias2, cores3, bias3, w_lin, b_lin)

    if _nc_cache is None:
        _nc_cache = _build_nc()
    nc = _nc_cache

    in_maps = []
    for i in range(N_CORES):
        m = {"x": x[i * B_CORE:(i + 1) * B_CORE]}
        m.update(w)
        in_maps.append(m)

    import os
    trace = bool(int(os.environ.get("KERNEL_TRACE", "0")))
    res = run_bass_kernel_spmd(
        nc, in_maps, core_ids=list(range(N_CORES)), trace=trace,
        trace_cores=[0] if trace else None,
    )
    global last_results
    last_results = res
    outs = [res.results[i]["y"].T for i in range(N_CORES)]   # [64, 10] each
    return np.concatenate(outs, axis=0).astype(np.float32)


last_results = None
